# revision 22
# baseline (speedup 1.0000x reference)
"""Bass/Trainium2 kernel for a 16-head causal MHA block with partial rotary.

Problem shapes (hardcoded): x [2,2048,1024] fp32, Wq/Wk/Wv/Wo [1024,1024],
mask = causal tril [2048,2048] (hardcoded causality; mask input unused).

Sharding over 8 NeuronCores: core c handles batch c//4 and the 4 heads
h0 = (c%4)*4 .. h0+3 (tensor parallel on heads).  Each core computes its
partial output y_h @ Wo[h-block] summed over its 4 heads; the host adds the
4 per-batch partials.

Device-side plan (per core) — fully software-pipelined over seq chunks:
  per seq tile st (128 rows):
    DMA x_st -> PE transposes (f32r, into a [128,1024] psum pair tile)
    -> xT chunk tile; v projection (8 accum matmuls) -> vt[st] (+ones col)
  per seq chunk sc (512 rows, after its 4 st tiles):
    q/k projections (dims,seq layout) with rotary fused into eviction;
    pt0 projections -> attention heads 0,1 -> pt1 projections -> heads 2,3
    attention: logits^T pair tiles [128 j x 1024] = k^T q (f32r), exp via
    ACT (scale 1/8, skipping fully-masked diag columns), causal zeroing of
    diagonal tiles (gpsimd affine_select), AV matmuls accumulate
    [65, 512] = [y^T ; colsums], normalize via reciprocal+broadcast.
    AV emission lags exp by `la` pairs (cross-block software pipeline).
    output projection out = y^T.T @ Wo per 128-row tile, DMA'd out.
Weight/const DMAs dispatch from the Pool queue (25ns/dispatch) so the x
stream on SP is never stuck behind them.
"""

import numpy as np
from collections import deque

S, D, H, HD, PROT = 2048, 1024, 16, 64, 32
NHC = 4            # heads per core
SEQT = S // 128    # 16
DCH = D // 128     # 8
NIC = 4            # i-chunks of 512

_CACHED = {}


def _rot_tables():
    invf = 10000.0 ** (-np.arange(0, PROT, 2, dtype=np.float64) / PROT)  # [16]
    ang = np.arange(S, dtype=np.float64)[None, :] * invf[:, None]        # [16, S]
    C64 = np.ones((64, S), np.float64)
    S64 = np.zeros((64, S), np.float64)
    for d in range(PROT):
        C64[d] = np.cos(ang[d // 2])
        S64[d] = (1.0 if d % 2 else -1.0) * np.sin(ang[d // 2])
    CT = np.concatenate([C64, C64], 0).astype(np.float32)
    ST = np.concatenate([S64, S64], 0).astype(np.float32)
    return CT, ST


def build_nc(reps=1, la=3, xpref=3, xpbufs=6, ebufs=4, pairb=2, ytb=2,
             ringb=2, rotb=3, obb=3, av='bf16', ablate=()):
    import concourse.bacc as bacc
    import concourse.mybir as mybir
    from concourse.tile import TileContext

    F32 = mybir.dt.float32
    F32R = mybir.dt.float32r
    AF = mybir.ActivationFunctionType
    ALU = mybir.AluOpType
    MMDT = F32R
    BF16 = mybir.dt.bfloat16
    F8 = mybir.dt.float8e4
    EDT = F8 if av == 'f8' else BF16
    DR = mybir.MatmulPerfMode.DoubleRow

    nc = bacc.Bacc("TRN2", target_bir_lowering=False, debug=False)

    x_d = nc.dram_tensor("x", [S, D], MMDT, kind="ExternalInput").ap()
    wq_d = nc.dram_tensor("wq", [128, 2048], MMDT, kind="ExternalInput").ap()
    wk_d = nc.dram_tensor("wk", [128, 2048], MMDT, kind="ExternalInput").ap()
    wv_d = nc.dram_tensor("wv", [128, 2048], MMDT, kind="ExternalInput").ap()
    wo_d = nc.dram_tensor("wo", [128, 2048], MMDT, kind="ExternalInput").ap()
    out_d = nc.dram_tensor("out", [S, D], F32, kind="ExternalOutput").ap()

    CT, ST = _rot_tables()
    ct_d = nc.inline_tensor(CT, "ct_const").ap()
    st_d = nc.inline_tensor(ST, "st_const").ap()
    id_d = nc.inline_tensor(np.eye(128, dtype=np.float32), "id_const").ap()
    ones_d = nc.inline_tensor(np.ones((128, NHC), np.float32), "ones_const").ap()
    import ml_dtypes
    ones8_d = nc.inline_tensor(
        np.ones((128, 2), ml_dtypes.float8_e4m3fn), "ones8_const").ap()

    SWAP_MASK = [i ^ 1 for i in range(32)]

    with TileContext(nc) as tc:
      for _rep in range(reps):
        with (
            tc.tile_pool(name="persist", bufs=1) as pp,
            tc.tile_pool(name="xp", bufs=xpbufs) as xp,
            tc.tile_pool(name="xTp", bufs=2) as xTp,
            tc.tile_pool(name="ep", bufs=ebufs) as epool,
            tc.tile_pool(name="yTp", bufs=4) as yTp,
            tc.tile_pool(name="obp", bufs=obb) as obp,
            tc.tile_pool(name="rotp", bufs=rotb) as rotp,
            tc.tile_pool(name="smallp", bufs=4) as smp,
            tc.tile_pool(name="psPair", bufs=pairb, space="PSUM") as psPair,
            tc.tile_pool(name="psYt", bufs=ytb, space="PSUM") as psYt,
            tc.tile_pool(name="psRing", bufs=ringb, space="PSUM") as psRing,
        ):
            # ---------------- persistent tiles + prelude DMAs --------------
            # weight/const DMAs dispatch from Pool's queue; x stream from SP
            ident = pp.tile([128, 128], MMDT, tag="ident")
            nc.sync.dma_start(out=ident[:], in_=id_d[:].bitcast(MMDT))

            xtiles = {}

            def start_x(sti):
                if sti < SEQT and sti not in xtiles:
                    t = xp.tile([128, D], MMDT, tag="x", name="x")
                    nc.sync.dma_start(
                        out=t[:], in_=x_d[sti * 128:(sti + 1) * 128, :])
                    xtiles[sti] = t

            start_x(0)
            wv_sb = pp.tile([128, 2048], MMDT, tag="wv_sb")
            nc.sync.dma_start(out=wv_sb[:], in_=wv_d[:])
            start_x(1)
            wq_sb = pp.tile([128, 2048], MMDT, tag="wq_sb")
            nc.sync.dma_start(out=wq_sb[:], in_=wq_d[:])
            start_x(2)
            wk_sb = pp.tile([128, 2048], MMDT, tag="wk_sb")
            nc.sync.dma_start(out=wk_sb[:], in_=wk_d[:])
            start_x(3)
            start_x(4)
            ct_sb = pp.tile([128, S], F32, tag="ct")
            st_sb = pp.tile([128, S], F32, tag="st")
            nc.sync.dma_start(out=ct_sb[:], in_=ct_d[:])
            start_x(5)
            nc.sync.dma_start(out=st_sb[:], in_=st_d[:])
            start_x(6)
            wo_sb = pp.tile([128, 2048], MMDT, tag="wo_sb")
            nc.sync.dma_start(out=wo_sb[:], in_=wo_d[:])
            start_x(7)
            ones_sb = pp.tile([128, NHC], F32, tag="ones_sb")
            nc.sync.dma_start(out=ones_sb[:], in_=ones_d[:])
            wv3 = wv_sb[:].rearrange("p (d c) -> p d c", d=DCH, c=256)
            wq3 = wq_sb[:].rearrange("p (d c) -> p d c", d=DCH, c=256)
            wk3 = wk_sb[:].rearrange("p (d c) -> p d c", d=DCH, c=256)
            wo3 = wo_sb[:].rearrange("p (t c) -> p t c", t=2, c=D)

            if av == 'f8':
                # fp8 DoubleRow AV: vt stored in st pairs [128, (t=2, h*64+d)]
                vt = [pp.tile([128, 2 * 256], F8, tag=f"vt{i}", name=f"vt{i}")
                      for i in range(SEQT // 2)]
                ones8 = pp.tile([128, 2], F8, tag="ones8")
                nc.vector.tensor_copy(ones8[:], ones_sb[:, 0:2])
                # e' = exp(l/8)/4 keeps e' under the e4m3 max of 448 while
                # staying out of denormals (cancels in y = num/den); bias
                # must be a per-partition AP
                ebias_sb = pp.tile([128, 1], F32, tag="ebias_sb")
                nc.gpsimd.memset(ebias_sb[:], float(-np.log(4.0)))
            else:
                # vt tiles persistent; ones columns written once up front
                vt = [pp.tile([128, NHC * 65], BF16, tag=f"vt{i}", name=f"vt{i}")
                      for i in range(SEQT)]
                for sti in range(SEQT):
                    vt_ones = vt[sti][:].rearrange(
                        "p (h c) -> p h c", h=NHC, c=65)[:, :, 64:65]
                    nc.vector.tensor_copy(
                        vt_ones, ones_sb[:].rearrange("p (h c) -> p h c", h=NHC, c=1))

            qT = {}   # (pt, sc) -> [128, 512] tile (rows = 2 heads x 64 dims)
            kT = {}
            yTt = {}  # (pt, ic) -> [128, 512] tile

            pend = deque()

            def drain(n):
                while len(pend) > n:
                    pend.popleft()()

            def get_yT(pt, ic):
                if (pt, ic) not in yTt:
                    yTt[(pt, ic)] = yTp.tile([128, 512], MMDT, tag="yT", name="yT")
                return yTt[(pt, ic)]

            def emit_block(ic, h):
                """Attention block (i-chunk ic, head h): pair-granular
                pipelined QK+exp+select (phase1) and AV+norm (phase2, lags
                by `la` pairs, cross-block).

                Diagonal j-tiles (jt >= 4*ic, mask offset w = jt*128-ic*512):
                cols [0,w) are fully masked -> never exp'd/read; cols
                [w,w+128) get a 128-col wedge select; cols >= w+128 are live.
                AV matmuls read only live columns (rhs col-restricted)."""
                pt, hh = h // 2, h % 2
                r0 = hh * 64
                njt = 4 * (ic + 1)
                state = {}
                for jp in range(njt // 2):
                    pair_ps = psPair.tile([128, 1024], F32, tag="pair", name="pair")
                    diag = 2 * jp >= 4 * ic       # both halves diagonal then
                    w0 = 2 * jp * 128 - ic * 512  # mask offset of half u=0
                    trimmed = diag and w0 >= 256  # tiles at offsets 256,384
                    qlo = 256 if trimmed else 0   # cols < qlo never read
                    for u in range(2):
                        jt = 2 * jp + u
                        if "qk" not in ablate:
                            nc.tensor.matmul(
                                pair_ps[:, u * 512 + qlo:(u + 1) * 512],
                                kT[(pt, jt // 4)][r0:r0 + 64,
                                                  (jt % 4) * 128:(jt % 4) * 128 + 128],
                                qT[(pt, ic)][r0:r0 + 64, qlo:512],
                                start=True, stop=True,
                            )
                    e = epool.tile([128, 1024], EDT, tag="e", name="e")
                    if "exp" in ablate:
                        nc.vector.tensor_copy(e[:], pair_ps[:])
                    elif trimmed:
                        nc.scalar.activation(
                            e[:, 256:512], pair_ps[:, 256:512], AF.Exp,
                            scale=0.125)
                        nc.scalar.activation(
                            e[:, 896:1024], pair_ps[:, 896:1024], AF.Exp,
                            scale=0.125)
                    else:
                        nc.scalar.activation(e[:], pair_ps[:], AF.Exp,
                                             scale=0.125)
                    if diag and "affine" not in ablate:
                        for u in range(2):
                            w = (2 * jp + u) * 128 - ic * 512
                            c0 = u * 512 + w
                            nc.gpsimd.affine_select(
                                out=e[:, c0:c0 + 128], in_=e[:, c0:c0 + 128],
                                compare_op=ALU.is_ge, fill=0.0,
                                base=0, channel_multiplier=-1,
                                pattern=[[1, 128]],
                            )

                    def phase2(jp=jp, e=e, diag=diag, trimmed=trimmed):
                        if jp == 0:
                            state["yt"] = psYt.tile([65, 512], F32, tag="yt",
                                                    name="yt")
                        yt = state["yt"]
                        if "av" not in ablate:
                            for u in range(2):
                                jt = 2 * jp + u
                                lo = min(512, max(0, jt * 128 - ic * 512))
                                nc.tensor.matmul(
                                    yt[:, lo:512],
                                    vt[jt][:, h * 65: h * 65 + 65],
                                    e[:, u * 512 + lo:(u + 1) * 512],
                                    start=(jt == 0), stop=(jt == njt - 1),
                                )
                        if jp == njt // 2 - 1 and "norm" not in ablate:
                            yU = smp.tile([65, 512], F32, tag="yU", name="yU",
                                          bufs=4)
                            nc.vector.tensor_copy(yU[:], yt[:])
                            rs = smp.tile([1, 512], F32, tag="rs", name="rs", bufs=2)
                            nc.vector.reciprocal(rs[0:1, :], yU[64:65, :])
                            bc = smp.tile([64, 512], F32, tag="bc", name="bc", bufs=2)
                            nc.gpsimd.partition_broadcast(bc[:], rs[0:1, :])
                            nc.vector.tensor_mul(
                                get_yT(pt, ic)[r0:r0 + 64, :], yU[0:64, :], bc[:])

                    pend.append(phase2)
                    drain(la)

            def emit_proj(name, w3, dstmap, pt, sc, xT3):
                """q/k projection + fused rotary for (tensor, pt, chunk sc)."""
                ps = psRing.tile([128, 512], F32, tag="ring", name="ring")
                for d in range(DCH):
                    nc.tensor.matmul(
                        ps[:],
                        w3[:, d, pt * 128:(pt + 1) * 128],
                        xT3[:, d, :],
                        start=(d == 0), stop=(d == DCH - 1),
                    )
                tile = pp.tile([128, 512], MMDT, tag=f"{name}T{pt}_{sc}",
                               name=f"{name}T{pt}_{sc}")
                dstmap[(pt, sc)] = tile
                if "rotary" in ablate:
                    nc.scalar.copy(out=tile[:], in_=ps[:])
                else:
                    t0 = rotp.tile([128, 512], F32, tag="t0", name="t0")
                    sw = rotp.tile([128, 512], F32, tag="sw", name="sw")
                    nc.vector.stream_shuffle(sw[:], ps[:], SWAP_MASK)
                    nc.vector.tensor_mul(
                        t0[:], ps[:], ct_sb[:, sc * 512:(sc + 1) * 512])
                    nc.gpsimd.tensor_mul(
                        sw[:], sw[:], st_sb[:, sc * 512:(sc + 1) * 512])
                    nc.gpsimd.tensor_add(tile[:], t0[:], sw[:])

            def make_oproj(ic, stl, dc):
                def run():
                    ps = psRing.tile([128, 512], F32, tag="ring", name="ring")
                    for pt in range(2):
                        nc.tensor.matmul(
                            ps[:],
                            yTt[(pt, ic)][:, stl * 128:(stl + 1) * 128],
                            wo3[:, pt, dc * 512:(dc + 1) * 512],
                            start=(pt == 0), stop=(pt == 1),
                        )
                    ob = obp.tile([128, 512], F32, tag="ob", name="ob")
                    nc.vector.tensor_copy(ob[:], ps[:])
                    nc.sync.dma_start(
                        out=out_d[(ic * 4 + stl) * 128:(ic * 4 + stl + 1) * 128,
                                  dc * 512:(dc + 1) * 512],
                        in_=ob[:],
                    )
                return run

            # ---------------- the pipelined main loop ----------------------
            xTt = None
            ntp = [0]  # transpose-evict round robin
            for sti in range(SEQT):
                start_x(sti + xpref)
                sc, stl = sti // 4, sti % 4
                if stl == 0:
                    xTt = xTp.tile([128, DCH * 512], MMDT, tag="xT", name="xT")
                xT3 = xTt[:].rearrange("p (d s) -> p d s", d=DCH, s=512)
                xt = xtiles.pop(sti)

                # A: 8 transposes into one [128,1024] pair-pool psum tile
                tp_ps = psPair.tile([128, 1024], F32, tag="pair", name="pair")
                for d in range(DCH):
                    nc.tensor.matmul(
                        tp_ps[:, d * 128:(d + 1) * 128].bitcast(MMDT),
                        xt[:, d * 128:(d + 1) * 128],
                        ident[:],
                        is_transpose=True, start=True, stop=True,
                    )
                dst = xT3[:, :, stl * 128: stl * 128 + 128]
                src = tp_ps[:].rearrange("p (d c) -> p d c", d=DCH, c=128)
                if ntp[0] % 2 == 0:
                    nc.vector.tensor_copy(dst, src)
                else:
                    nc.scalar.copy(out=dst, in_=src)
                ntp[0] += 1

                # V: projection for this seq tile
                vps = psRing.tile([128, 512], F32, tag="ring", name="ring")
                for d in range(DCH):
                    nc.tensor.matmul(
                        vps[:, :256],
                        xT3[:, d, stl * 128: stl * 128 + 128],
                        wv3[:, d, :],
                        start=(d == 0), stop=(d == DCH - 1),
                    )
                if av == 'f8':
                    dst = vt[sti // 2][:].rearrange(
                        "p (t c) -> p t c", t=2, c=256)[:, sti % 2, :]
                    nc.vector.tensor_copy(dst, vps[:, :256])
                else:
                    dst = vt[sti][:].rearrange(
                        "p (h c) -> p h c", h=NHC, c=65)[:, :, :64]
                    src = vps[:, :256].rearrange("p (h c) -> p h c", h=NHC, c=64)
                    nc.vector.tensor_copy(dst, src)

                if stl == 3:
                    emit_proj("q", wq3, qT, 0, sc, xT3)
                    emit_proj("k", wk3, kT, 0, sc, xT3)
                    emit_proj("q", wq3, qT, 1, sc, xT3)
                    emit_proj("k", wk3, kT, 1, sc, xT3)
                    # previous chunk's AV tail + O parts fill the PE queue
                    # while this chunk's rotary chains complete
                    drain(0)
                    for h in range(NHC):
                        emit_block(sc, h)
                    # output projection parts, deferred into the pipeline
                    for stl2 in range(4):
                        for dc in range(2):
                            pend.append(make_oproj(sc, stl2, dc))

            drain(0)

    nc.compile()
    return nc


def _in_maps(x, Wq, Wk, Wv, Wo):
    maps = []
    for core in range(8):
        b, hg = core // 4, core % 4
        c0 = hg * 4 * HD
        def pack_w(W):   # [1024, 256] -> [128, 8*256] (d-chunk major cols)
            return np.ascontiguousarray(
                W.reshape(8, 128, 256).transpose(1, 0, 2).reshape(128, 2048))

        maps.append({
            "x": np.ascontiguousarray(x[b]),
            "wq": pack_w(Wq[:, c0:c0 + 256]),
            "wk": pack_w(Wk[:, c0:c0 + 256]),
            "wv": pack_w(Wv[:, c0:c0 + 256]),
            "wo": np.ascontiguousarray(
                Wo[c0:c0 + 256, :].reshape(2, 128, 1024)
                .transpose(1, 0, 2).reshape(128, 2048)),
        })
    return maps


def kernel(x, mask, Wq, Wk, Wv, Wo):
    from concourse.bass_utils import run_bass_kernel_spmd

    x, Wq, Wk, Wv, Wo = (np.asarray(a, np.float32) for a in (x, Wq, Wk, Wv, Wo))
    if "nc" not in _CACHED:
        _CACHED["nc"] = build_nc()
    res = run_bass_kernel_spmd(_CACHED["nc"], _in_maps(x, Wq, Wk, Wv, Wo),
                               core_ids=list(range(8)))
    out = np.zeros((2, S, D), np.float32)
    for core in range(8):
        out[core // 4] += res.results[core]["out"]
    return out
